# revision 1
# baseline (speedup 1.0000x reference)
"""Trainium2 Bass kernel for a 2-layer GAT model (GATConv -> ELU -> GATConv -> ELU
-> mean readout -> linear).

Strategy (8 NeuronCores, SPMD):
  - Partition dst nodes (and their incoming edges) across the 8 cores.
  - Each core computes the dense projection table for its node shard; one
    AllGather per layer replicates it. Rows are packed head-major as
    8 x [32 h-channels | a_src.h] (264 bf16) so the gathered edge rows are
    contiguous for the DVE fast modes; a_dst.h lives in a separate [SR, 8]
    table that stays core-local.
  - Edges (sorted by dst, packed into <=128-node / 16-chunk blocks) are
    processed 128 per chunk: one single-index-per-partition indirect DMA
    fetches the source rows (the HW DGE only supports one index per
    partition); the one-hot selector S (edges x slots) is built with a
    tensor_scalar is_equal; S^T (via PE transpose) broadcasts the block's
    a_dst values to edges with a tiny matmul; scores exp(leaky_relu(as+ad))
    are expanded head->channel on the Scalar engine; the weighted aggregation
    and the softmax denominator accumulate in PSUM via selector matmuls.
  - The epilogue divides by denom (+1e-16), adds bias, applies ELU, and either
    scatters rows back to DRAM (layer-1 output -> layer-2 input) or accumulates
    the column sum for the mean readout (layer 2).
  - A tiny AllReduce combines the per-core column sums; every core finishes the
    linear head redundantly and writes the [1] output.

All graph-dependent tables (gather indices, one-hot slot ids, scatter targets)
are built host-side in numpy; all model FLOPs run on the Trainium cores.
"""

import sys

import numpy as np

for _p in ("/opt/trn_rl_repo",):
    if _p not in sys.path:
        sys.path.insert(0, _p)

from concourse import bass, mybir, tile  # noqa: E402
from concourse.bass import IndirectOffsetOnAxis  # noqa: E402
from concourse.bass_utils import run_bass_kernel_spmd  # noqa: E402
from concourse.masks import make_identity  # noqa: E402

F32 = mybir.dt.float32
BF16 = mybir.dt.bfloat16
I32 = mybir.dt.int32
NP_BF16 = mybir.dt.np(BF16)

N_CORES = 8
NEG_SLOPE = 0.2
NEG_BIG = -1e30


# ----------------------------------------------------------------------------
# Host-side graph preprocessing
# ----------------------------------------------------------------------------
def _preprocess(edge_index: np.ndarray, n_nodes: int, n_cores: int, k_ch: int):
    """Partition edges by dst shard; build per-core block/chunk tables."""
    src = np.asarray(edge_index[0], dtype=np.int64)
    dst = np.asarray(edge_index[1], dtype=np.int64)
    nsh = (n_nodes + n_cores - 1) // n_cores
    sr = ((nsh + 1 + 127) // 128) * 128
    sent = sr - 1

    owner = np.minimum(dst // nsh, n_cores - 1)
    src_owner = np.minimum(src // nsh, n_cores - 1)
    src_grow = src_owner * sr + (src - src_owner * nsh)

    cores = []
    max_blocks = 0
    for k in range(n_cores):
        lo, hi = k * nsh, min((k + 1) * nsh, n_nodes)
        n_local = hi - lo
        m = owner == k
        e_dst = dst[m] - lo
        e_srcg = src_grow[m]
        order = np.argsort(e_dst, kind="stable")
        e_dst = e_dst[order]
        e_srcg = e_srcg[order]
        deg = np.bincount(e_dst, minlength=n_local)
        blocks = []
        cap = k_ch * 128
        v0 = e0 = cur_e = cur_n = 0
        for v in range(n_local):
            d = int(deg[v])
            if cur_n + 1 > 128 or cur_e + d > cap:
                blocks.append((v0, cur_n, e0, cur_e))
                v0, e0 = v, e0 + cur_e
                cur_e = cur_n = 0
            cur_e += d
            cur_n += 1
        blocks.append((v0, cur_n, e0, cur_e))
        cores.append(dict(blocks=blocks, e_dst=e_dst, e_srcg=e_srcg))
        max_blocks = max(max_blocks, len(blocks))

    B = max_blocks
    C = B * k_ch
    src_g = np.zeros((n_cores, 128, C), dtype=np.int32)
    dst_rel = np.zeros((n_cores, 128, C), dtype=np.float32)
    scat = np.zeros((n_cores, 128, B), dtype=np.int32)
    for k in range(n_cores):
        info = cores[k]
        my_sent = k * sr + sent
        src_g[k, :, :] = my_sent
        scat[k, :, :] = sent  # dump row
        for b, (v0, nv, e0, ne) in enumerate(info["blocks"]):
            scat[k, :nv, b] = v0 + np.arange(nv)
            es = slice(e0, e0 + ne)
            eg = info["e_srcg"][es]
            rel = (info["e_dst"][es] - v0).astype(np.float32)
            j = np.arange(ne)
            ch = b * k_ch + (j // 128)
            p = j % 128
            src_g[k, p, ch] = eg
            dst_rel[k, p, ch] = rel
    return dict(SR=sr, B=B, C=C, NSH=nsh, sent=sent,
                src_g=src_g, dst_rel=dst_rel, scat=scat)


LEGALIZE_WAITS = True  # sim_test disables: CoreSim's race detector can't track
                       # the synthetic EventSemaphore waits (HW/walrus needs them)


def _legalize_waits(nc, cap=1):
    """Split multi-wait instructions: the TRN2 engine-instruction encodings hold
    only a limited number of sync-wait commands (walrus: "Too many sync wait
    commands"). Move excess waits onto standalone sequencer EventSemaphore
    instructions inserted just before, on the same engine queue."""
    for bb in nc.main_func.blocks:
        out = []
        n_split = 0
        for ins in bb.instructions:
            si = ins.sync_info
            waits = list(si.on_wait) if si and si.on_wait else []
            if len(waits) <= cap:
                out.append(ins)
                continue
            movable = [
                w for w in waits
                if w.sync_type == "semaphore" and w.wait_mode == "sem-ge-imm"
            ]
            keep = [w for w in waits if w not in movable]
            n_move = min(len(movable), len(waits) - cap)
            for wt in movable[:n_move]:
                ev = mybir.InstEventSemaphore(
                    name=f"{ins.name}-w{n_split}", ins=[], outs=[]
                )
                n_split += 1
                ev.engine = ins.engine
                ev.sync_info = mybir.SyncInfo(on_wait=[wt], on_update=[])
                out.append(ev)
            keep.extend(movable[n_move:])
            ins.sync_info = mybir.SyncInfo(
                on_wait=keep, on_update=list(si.on_update) if si.on_update else []
            )
            out.append(ins)
        bb.instructions = out


# ----------------------------------------------------------------------------
# Bass program
# ----------------------------------------------------------------------------
def _build_program(cfg):
    SR, B, C, K_CH = cfg["SR"], cfg["B"], cfg["C"], cfg["K_CH"]
    F = cfg["F"]            # input features
    D = cfg["D"]            # hidden = heads*chan
    H = cfg["H"]            # heads
    CH = D // H             # channels per head
    RW = D + H              # packed row width: H x [CH | as] (264)
    HW_ = RW + H            # dense psum width: RW + ad (272)
    G = N_CORES * SR
    n_tiles = SR // 128
    kd = max(1, D // 128)   # K-tiles for layer-2 dense
    use_bias = cfg["use_bias"]

    nc = bass.Bass()

    x1T = nc.declare_dram_parameter("x1T", [F, SR], BF16, isOutput=False)
    srcg_p = nc.declare_dram_parameter("src_g", [128, C], I32, isOutput=False)
    drel_p = nc.declare_dram_parameter("dst_rel", [128, C], F32, isOutput=False)
    scat_p = nc.declare_dram_parameter("scat", [128, B], I32, isOutput=False)
    w1e_p = nc.declare_dram_parameter("W1e", [F, HW_], BF16, isOutput=False)
    w2e_p = nc.declare_dram_parameter("W2e", [D, HW_], BF16, isOutput=False)
    iota_p = nc.declare_dram_parameter("iota_row", [128, 128], BF16, isOutput=False)
    sent_p = nc.declare_dram_parameter("sent_row", [1, RW], BF16, isOutput=False)
    lwg_p = nc.declare_dram_parameter("linw_g", [1, D], F32, isOutput=False)
    lwuw_p = nc.declare_dram_parameter("linw_uw", [1, 2], F32, isOutput=False)
    uw_p = nc.declare_dram_parameter("uw", [1, 2], F32, isOutput=False)
    lb_p = nc.declare_dram_parameter("lin_b", [1, 1], F32, isOutput=False)
    if use_bias:
        b1_p = nc.declare_dram_parameter("bias1r", [128, D], F32, isOutput=False)
        b2_p = nc.declare_dram_parameter("bias2r", [128, D], F32, isOutput=False)
    out_p = nc.declare_dram_parameter("out", [1, 1], F32, isOutput=True)

    hext_own = [nc.dram_tensor(f"hext{i}_own", [SR, RW], BF16) for i in (1, 2)]
    ad_own = [nc.dram_tensor(f"ad{i}_own", [SR, H], BF16) for i in (1, 2)]
    hext_full = [
        nc.dram_tensor(f"hext{i}_full", [G, RW], BF16, addr_space="Shared")
        for i in (1, 2)
    ]
    x2_dram = nc.dram_tensor("x2", [SR, D], BF16)
    cs_in = nc.dram_tensor("cs_in", [1, D], F32)
    cs_out = nc.dram_tensor("cs_out", [1, D], F32, addr_space="Shared")

    rg = [list(range(N_CORES))]

    with tile.TileContext(nc) as tc:
        with (
            tc.tile_pool(name="const", bufs=1) as cp,
            tc.tile_pool(name="dstg", bufs=3) as dstgp,
            tc.tile_pool(name="gblk", bufs=4) as gp,
            tc.tile_pool(name="adb", bufs=2) as adp,
            tc.tile_pool(name="sc", bufs=2) as scp,
            tc.tile_pool(name="sce", bufs=2) as sep,
            tc.tile_pool(name="sS", bufs=2) as sp_,
            tc.tile_pool(name="ep", bufs=2) as epp,
            tc.tile_pool(name="x2s", bufs=3) as x2p,
            tc.tile_pool(name="fin", bufs=1) as fp_,
            tc.tile_pool(name="psA", bufs=2, space="PSUM") as psA,
            tc.tile_pool(name="psD", bufs=2, space="PSUM") as psD,
            tc.tile_pool(name="psT", bufs=1, space="PSUM") as psT,
            tc.tile_pool(name="psAD", bufs=2, space="PSUM") as psAD,
            tc.tile_pool(name="psC", bufs=1, space="PSUM") as psC,
        ):
            # ---- constants -------------------------------------------------
            x1T_sb = cp.tile([F, SR], BF16, tag="x1T")
            nc.sync.dma_start(out=x1T_sb[:], in_=x1T[:])
            srcg_sb = cp.tile([128, C], I32, tag="srcg")
            nc.sync.dma_start(out=srcg_sb[:], in_=srcg_p[:])
            drel_sb = cp.tile([128, C], F32, tag="drel")
            nc.sync.dma_start(out=drel_sb[:], in_=drel_p[:])
            scat_sb = cp.tile([128, B], I32, tag="scat")
            nc.sync.dma_start(out=scat_sb[:], in_=scat_p[:])
            w1e_sb = cp.tile([F, HW_], BF16, tag="w1e")
            nc.sync.dma_start(out=w1e_sb[:], in_=w1e_p[:])
            w2e_sb = []
            for q in range(kd):
                wt = cp.tile([128, HW_], BF16, tag=f"w2e{q}")
                nc.sync.dma_start(out=wt[:], in_=w2e_p[q * 128:(q + 1) * 128, :])
                w2e_sb.append(wt)
            iota_sb = cp.tile([128, 128], BF16, tag="iota")
            nc.sync.dma_start(out=iota_sb[:], in_=iota_p[:])
            sent_sb = cp.tile([1, RW], BF16, tag="sent")
            nc.sync.dma_start(out=sent_sb[:], in_=sent_p[:])
            ident_sb = cp.tile([128, 128], BF16, tag="ident")
            make_identity(nc, ident_sb[:])
            ones_sb = cp.tile([128, 1], BF16, tag="ones")
            nc.vector.memset(ones_sb[:], 1.0)
            adsent_sb = cp.tile([1, H], BF16, tag="adsent")
            nc.vector.memset(adsent_sb[:], 0.0)
            lwg_sb = cp.tile([1, D], F32, tag="lwg")
            nc.sync.dma_start(out=lwg_sb[:], in_=lwg_p[:])
            lwuw_sb = cp.tile([1, 2], F32, tag="lwuw")
            nc.sync.dma_start(out=lwuw_sb[:], in_=lwuw_p[:])
            uw_sb = cp.tile([1, 2], F32, tag="uw")
            nc.sync.dma_start(out=uw_sb[:], in_=uw_p[:])
            lb_sb = cp.tile([1, 1], F32, tag="lb")
            nc.sync.dma_start(out=lb_sb[:], in_=lb_p[:])
            if use_bias:
                b1_sb = cp.tile([128, D], F32, tag="b1")
                nc.sync.dma_start(out=b1_sb[:], in_=b1_p[:])
                b2_sb = cp.tile([128, D], F32, tag="b2")
                nc.sync.dma_start(out=b2_sb[:], in_=b2_p[:])

            # zero x2 padding rows (read by the transpose, never scattered)
            zpad = SR - cfg["NSH"] - 1
            if zpad > 0:
                zt = cp.tile([128, D], BF16, tag="zpad")
                nc.vector.memset(zt[:], 0.0)
                nc.sync.dma_start(
                    out=x2_dram[cfg["NSH"]: cfg["NSH"] + zpad, :], in_=zt[0:zpad, :]
                )

            csum_ps = psC.tile([1, D], F32, tag="cs")
            x2T_sb = None

            for layer in range(2):
                we_sb = w1e_sb if layer == 0 else None
                # ---- dense ------------------------------------------------
                for t in range(n_tiles):
                    ps = psA.tile([128, HW_], F32, tag="ps")
                    if layer == 0:
                        nc.tensor.matmul(
                            out=ps[:],
                            lhsT=x1T_sb[:, t * 128:(t + 1) * 128],
                            rhs=we_sb[:],
                            start=True, stop=True,
                        )
                    else:
                        for q in range(kd):
                            nc.tensor.matmul(
                                out=ps[:],
                                lhsT=x2T_sb[q][:, t * 128:(t + 1) * 128],
                                rhs=w2e_sb[q][:],
                                start=(q == 0), stop=(q == kd - 1),
                            )
                    stg = dstgp.tile([128, HW_], BF16, tag="stg")
                    nc.vector.tensor_copy(out=stg[:], in_=ps[:])
                    if t == n_tiles - 1:
                        nc.sync.dma_start(
                            out=hext_own[layer][t * 128: SR - 1, :],
                            in_=stg[0:127, 0:RW],
                        )
                        nc.sync.dma_start(
                            out=hext_own[layer][SR - 1: SR, :], in_=sent_sb[:]
                        )
                        nc.sync.dma_start(
                            out=ad_own[layer][t * 128: SR - 1, :],
                            in_=stg[0:127, RW:HW_],
                        )
                        nc.sync.dma_start(
                            out=ad_own[layer][SR - 1: SR, :], in_=adsent_sb[:]
                        )
                    else:
                        nc.sync.dma_start(
                            out=hext_own[layer][t * 128:(t + 1) * 128, :],
                            in_=stg[:, 0:RW],
                        )
                        nc.sync.dma_start(
                            out=ad_own[layer][t * 128:(t + 1) * 128, :],
                            in_=stg[:, RW:HW_],
                        )

                # ---- AllGather --------------------------------------------
                nc.gpsimd.collective_compute(
                    "AllGather",
                    mybir.AluOpType.bypass,
                    ins=[hext_own[layer][:]],
                    outs=[hext_full[layer][:]],
                    replica_groups=rg,
                )

                # ---- edge pass --------------------------------------------
                for b in range(B):
                    c0 = b * K_CH
                    gblk = gp.tile([128, K_CH * RW], BF16, tag="gblk")
                    for kk in range(K_CH):
                        nc.gpsimd.indirect_dma_start(
                            out=gblk[:, kk * RW:(kk + 1) * RW],
                            out_offset=None,
                            in_=hext_full[layer][:],
                            in_offset=IndirectOffsetOnAxis(
                                ap=srcg_sb[:, c0 + kk:c0 + kk + 1], axis=0
                            ),
                        )
                    adb = adp.tile([128, H], BF16, tag="adb")
                    nc.gpsimd.indirect_dma_start(
                        out=adb[:],
                        out_offset=None,
                        in_=ad_own[layer][:],
                        in_offset=IndirectOffsetOnAxis(
                            ap=scat_sb[:, b:b + 1], axis=0
                        ),
                    )
                    # S (edges x slots) and S^T per chunk (S^T copies 4-batched)
                    s_all = sp_.tile([128, K_CH * 128], BF16, tag="s_all")
                    st_all = sp_.tile([128, K_CH * 128], BF16, tag="st_all")
                    ps_ad = psAD.tile([128, K_CH * H], F32, tag="ps_ad")
                    for kk in range(K_CH):
                        ssl = s_all[:, kk * 128:(kk + 1) * 128]
                        nc.vector.tensor_scalar(
                            out=ssl,
                            in0=iota_sb[:],
                            scalar1=drel_sb[:, c0 + kk:c0 + kk + 1],
                            scalar2=None,
                            op0=mybir.AluOpType.is_equal,
                        )
                    for q4 in range(0, K_CH, 4):
                        nq = min(4, K_CH - q4)
                        pst = psT.tile([128, 4 * 128], BF16, tag="pst")
                        for j in range(nq):
                            kk = q4 + j
                            nc.tensor.transpose(
                                out=pst[:, j * 128:(j + 1) * 128],
                                in_=s_all[:, kk * 128:(kk + 1) * 128],
                                identity=ident_sb[:],
                            )
                        nc.vector.tensor_copy(
                            out=st_all[:, q4 * 128:(q4 + nq) * 128],
                            in_=pst[:, 0:nq * 128],
                        )
                        for j in range(nq):
                            kk = q4 + j
                            nc.tensor.matmul(
                                out=ps_ad[:, kk * H:(kk + 1) * H],
                                lhsT=st_all[:, kk * 128:(kk + 1) * 128],
                                rhs=adb[:],
                                start=True, stop=True,
                            )
                    # scores: e = leaky(as + ad); expe expanded to channels
                    adc = scp.tile([128, K_CH * H], BF16, tag="adc")
                    nc.vector.tensor_copy(out=adc[:], in_=ps_ad[:])
                    scc = scp.tile([128, K_CH * H], BF16, tag="scc")
                    as_view = gblk[:].rearrange("p (g w) -> p g w", w=RW)[
                        :, :, :
                    ].rearrange("p g (h c) -> p g h c", c=CH + 1)[:, :, :, CH]
                    nc.vector.tensor_tensor(
                        out=scc[:].rearrange("p (g h) -> p g h", h=H),
                        in0=as_view,
                        in1=adc[:].rearrange("p (g h) -> p g h", h=H),
                        op=mybir.AluOpType.add,
                    )
                    t2 = scp.tile([128, K_CH * H], BF16, tag="t2")
                    nc.vector.tensor_scalar_mul(out=t2[:], in0=scc[:], scalar1=NEG_SLOPE)
                    nc.vector.tensor_tensor(
                        out=scc[:], in0=scc[:], in1=t2[:], op=mybir.AluOpType.max
                    )
                    sce = sep.tile([128, K_CH * RW], BF16, tag="sce")
                    nc.scalar.activation(
                        out=sce[:],
                        in_=scc[:].rearrange("p (g h) -> p g h", h=H)
                        .unsqueeze(-1).to_broadcast([128, K_CH, H, CH + 1]),
                        func=mybir.ActivationFunctionType.Exp,
                    )
                    # weight gathered rows by expe (single contiguous 4x op)
                    nc.vector.tensor_tensor(
                        out=gblk[:], in0=gblk[:], in1=sce[:],
                        op=mybir.AluOpType.mult,
                    )
                    # aggregation + denominator
                    ps_o = psA.tile([128, HW_], F32, tag="ps")
                    ps_d = psD.tile([128, H], F32, tag="psd")
                    for kk in range(K_CH):
                        first, last = kk == 0, kk == K_CH - 1
                        lhsT = s_all[:, kk * 128:(kk + 1) * 128]
                        nc.tensor.matmul(
                            out=ps_o[:, 0:RW],
                            lhsT=lhsT,
                            rhs=gblk[:, kk * RW:(kk + 1) * RW],
                            start=first, stop=last,
                        )
                        nc.tensor.matmul(
                            out=ps_d[:],
                            lhsT=lhsT,
                            rhs=sce[:, kk * RW:kk * RW + RW].rearrange(
                                "p (h c) -> p h c", c=CH + 1
                            )[:, :, 0],
                            start=first, stop=last,
                        )
                    # ---- epilogue ----------------------------------------
                    rd = epp.tile([128, H], F32, tag="rd")
                    nc.vector.tensor_scalar_add(out=rd[:], in0=ps_d[:], scalar1=1e-16)
                    nc.vector.reciprocal(out=rd[:], in_=rd[:])
                    rde = epp.tile([128, D], F32, tag="rde")
                    nc.scalar.activation(
                        out=rde[:],
                        in_=rd[:].unsqueeze(-1).to_broadcast([128, H, CH]),
                        func=mybir.ActivationFunctionType.Copy,
                    )
                    st = epp.tile([128, D], F32, tag="st")
                    nc.vector.tensor_tensor(
                        out=st[:].rearrange("p (h c) -> p h c", c=CH),
                        in0=ps_o[:, 0:RW].rearrange(
                            "p (h c) -> p h c", c=CH + 1
                        )[:, :, 0:CH],
                        in1=rde[:].rearrange("p (h c) -> p h c", c=CH),
                        op=mybir.AluOpType.mult,
                    )
                    if use_bias:
                        bsb = b1_sb if layer == 0 else b2_sb
                        nc.vector.tensor_tensor(
                            out=st[:], in0=st[:], in1=bsb[:], op=mybir.AluOpType.add
                        )
                    tm = epp.tile([128, D], F32, tag="tm")
                    nc.vector.tensor_scalar_min(out=tm[:], in0=st[:], scalar1=0.0)
                    nc.scalar.activation(
                        out=tm[:], in_=tm[:], func=mybir.ActivationFunctionType.Exp
                    )
                    nc.vector.tensor_scalar(
                        out=st[:], in0=st[:],
                        scalar1=0.0, scalar2=-1.0,
                        op0=mybir.AluOpType.max, op1=mybir.AluOpType.add,
                    )
                    xs = x2p.tile([128, D], BF16, tag="xs")
                    nc.vector.tensor_tensor(
                        out=xs[:], in0=st[:], in1=tm[:], op=mybir.AluOpType.add
                    )
                    if layer == 0:
                        nc.gpsimd.indirect_dma_start(
                            out=x2_dram[:],
                            out_offset=IndirectOffsetOnAxis(
                                ap=scat_sb[:, b:b + 1], axis=0
                            ),
                            in_=xs[:],
                            in_offset=None,
                        )
                    else:
                        nc.tensor.matmul(
                            out=csum_ps[:],
                            lhsT=ones_sb[:],
                            rhs=xs[:],
                            start=(b == 0), stop=(b == B - 1),
                        )

                # ---- transpose x2 for layer-2 dense lhsT -------------------
                if layer == 0:
                    x2T_sb = []
                    for q in range(kd):
                        xt = cp.tile([128, SR], BF16, tag=f"x2T{q}")
                        nc.sync.dma_start_transpose(
                            out=xt[:], in_=x2_dram[:, q * 128:(q + 1) * 128]
                        )
                        x2T_sb.append(xt)

            # ---- readout ---------------------------------------------------
            cs_sb = fp_.tile([1, D], F32, tag="cs_sb")
            nc.vector.tensor_copy(out=cs_sb[:], in_=csum_ps[:])
            nc.sync.dma_start(out=cs_in[:], in_=cs_sb[:])
            nc.gpsimd.collective_compute(
                "AllReduce",
                mybir.AluOpType.add,
                ins=[cs_in[:]],
                outs=[cs_out[:]],
                replica_groups=rg,
            )
            cs2 = fp_.tile([1, D], F32, tag="cs2")
            nc.sync.dma_start(out=cs2[:], in_=cs_out[:])
            tg = fp_.tile([1, D], F32, tag="tg")
            acc1 = fp_.tile([1, 1], F32, tag="acc1")
            nc.vector.tensor_tensor(
                out=tg[:], in0=cs2[:], in1=lwg_sb[:], op=mybir.AluOpType.mult
            )
            nc.vector.tensor_reduce(
                out=acc1[:], in_=tg[:], axis=mybir.AxisListType.X,
                op=mybir.AluOpType.add,
            )
            t2f = fp_.tile([1, 2], F32, tag="t2f")
            acc2 = fp_.tile([1, 1], F32, tag="acc2")
            nc.vector.tensor_tensor(
                out=t2f[:], in0=uw_sb[:], in1=lwuw_sb[:], op=mybir.AluOpType.mult
            )
            nc.vector.tensor_reduce(
                out=acc2[:], in_=t2f[:], axis=mybir.AxisListType.X,
                op=mybir.AluOpType.add,
            )
            nc.vector.tensor_tensor(
                out=acc1[:], in0=acc1[:], in1=acc2[:], op=mybir.AluOpType.add
            )
            nc.vector.tensor_tensor(
                out=acc1[:], in0=acc1[:], in1=lb_sb[:], op=mybir.AluOpType.add
            )
            nc.sync.dma_start(out=out_p[:], in_=acc1[:])

    if LEGALIZE_WAITS:
        _legalize_waits(nc)
    return nc


# ----------------------------------------------------------------------------
# Host-side input assembly
# ----------------------------------------------------------------------------
def _att_matrix(att: np.ndarray) -> np.ndarray:
    Hh, Cc = att.shape
    A = np.zeros((Hh * Cc, Hh), dtype=np.float64)
    for h in range(Hh):
        A[h * Cc:(h + 1) * Cc, h] = att[h]
    return A


def _pack_we(W, a_s, a_d, H, CH):
    """[W | W@As | W@Ad] -> packed [ per-head (W_cols | as_col) x H | ad ]."""
    K = W.shape[0]
    Ms = W @ _att_matrix(a_s)   # [K, H]
    Md = W @ _att_matrix(a_d)   # [K, H]
    out = np.zeros((K, H * (CH + 1) + H), dtype=np.float64)
    for h in range(H):
        out[:, h * (CH + 1): h * (CH + 1) + CH] = W[:, h * CH:(h + 1) * CH]
        out[:, h * (CH + 1) + CH] = Ms[:, h]
    out[:, H * (CH + 1):] = Md
    return out


def _make_inputs(prep, cfg, x, u, w, W1, as1, ad1, b1, W2, as2, ad2, b2,
                 lin_w, lin_b):
    SR, NSH = prep["SR"], prep["NSH"]
    F, D, H = cfg["F"], cfg["D"], cfg["H"]
    CH = D // H
    RW = D + H
    n_nodes = x.shape[0]

    W1e = _pack_we(W1, as1, ad1, H, CH).astype(NP_BF16)
    W2e = _pack_we(W2, as2, ad2, H, CH).astype(NP_BF16)
    iota_row = np.tile(np.arange(128, dtype=np.float32), (128, 1)).astype(NP_BF16)
    sent_row = np.zeros((1, RW), dtype=np.float32)
    for h in range(H):
        sent_row[0, h * (CH + 1) + CH] = NEG_BIG
    linw_g = (lin_w[0, :D] / float(n_nodes)).astype(np.float32).reshape(1, D)
    linw_uw = lin_w[0, D:D + 2].astype(np.float32).reshape(1, 2)
    uwv = np.array([[float(u), float(w)]], dtype=np.float32)
    lbv = np.asarray(lin_b, dtype=np.float32).reshape(1, 1)

    in_maps = []
    for k in range(N_CORES):
        lo = k * NSH
        hi = min(lo + NSH, n_nodes)
        xs = np.zeros((SR, F), dtype=np.float32)
        xs[: hi - lo] = x[lo:hi]
        m = {
            "x1T": np.ascontiguousarray(xs.T).astype(NP_BF16),
            "src_g": prep["src_g"][k],
            "dst_rel": prep["dst_rel"][k],
            "scat": prep["scat"][k],
            "W1e": W1e,
            "W2e": W2e,
            "iota_row": iota_row,
            "sent_row": sent_row.astype(NP_BF16),
            "linw_g": linw_g,
            "linw_uw": linw_uw,
            "uw": uwv,
            "lin_b": lbv,
        }
        if cfg["use_bias"]:
            m["bias1r"] = np.tile(b1.astype(np.float32), (128, 1))
            m["bias2r"] = np.tile(b2.astype(np.float32), (128, 1))
        in_maps.append(m)
    return in_maps


def build_all(x, edge_index, u, w, W1, att_src1, att_dst1, bias1,
              W2, att_src2, att_dst2, bias2, lin_w, lin_b, k_ch=16, debug=False):
    n_nodes, F = x.shape
    H, Cc = att_src1.shape
    D = H * Cc
    use_bias = bool(np.any(bias1) or np.any(bias2))
    prep = _preprocess(np.asarray(edge_index), n_nodes, N_CORES, k_ch)
    cfg = dict(
        SR=prep["SR"], B=prep["B"], C=prep["C"], NSH=prep["NSH"], K_CH=k_ch,
        F=F, D=D, H=H, use_bias=use_bias, debug=debug,
    )
    nc = _build_program(cfg)
    in_maps = _make_inputs(
        prep, cfg, np.asarray(x, np.float32), u, w,
        np.asarray(W1, np.float64), np.asarray(att_src1, np.float64),
        np.asarray(att_dst1, np.float64), np.asarray(bias1, np.float64),
        np.asarray(W2, np.float64), np.asarray(att_src2, np.float64),
        np.asarray(att_dst2, np.float64), np.asarray(bias2, np.float64),
        np.asarray(lin_w, np.float64), np.asarray(lin_b, np.float64),
    )
    return nc, in_maps


def kernel(**inputs) -> np.ndarray:
    nc, in_maps = build_all(
        inputs["x"], inputs["edge_index"], inputs["u"], inputs["w"],
        inputs["W1"], inputs["att_src1"], inputs["att_dst1"], inputs["bias1"],
        inputs["W2"], inputs["att_src2"], inputs["att_dst2"], inputs["bias2"],
        inputs["lin_w"], inputs["lin_b"],
    )
    res = run_bass_kernel_spmd(nc, in_maps, core_ids=list(range(N_CORES)))
    return res.results[0]["out"].reshape(1).astype(np.float32)

